# revision 38
# baseline (speedup 1.0000x reference)
"""Multi-head self-attention (B=2, S=2048, D=1024, H=16) on 8 TRN2 NeuronCores.

Sharding: core c handles batch b = c//4 and head group g = c%4 (4 heads each).
Each core computes qkv projection for its heads, masked-softmax attention, and
a partial output projection; the host sums the 4 partial outputs per batch.

Scores are computed transposed (keys on partitions, queries on the free dim) so
the P^T tile the PV matmul needs comes straight out of exp() with no transpose.
Softmax skips max-subtraction (scores are O(1) here); the denominator falls out
of a ones-column appended to the V stationary.

Mask handling: each 128key x 512query score tile is classified host-side as
skip / full / partial.  For partial tiles the leading fully-masked query
columns are sliced off the matmul moving dim entirely (for the causal mask
this removes all wasted columns), and only the remaining "mixed" region
(127 cols for causal - one shared pattern) is multiplied by a 0/1 mask after
exp.  Inputs are passed pre-tiled ([128, ...] partition-major) so every DMA is
a handful of large descriptors, chunked and spread over three engines so the
PE starts ~4us after the DMA stream does.  Scores and PV matmuls are
software-pipelined one tile-group apart so the PE is never waiting on exp.
"""

from contextlib import ExitStack

import numpy as np

import concourse.bass as bass
import concourse.tile as tile
from concourse import bacc, mybir
from concourse.bass_utils import run_bass_kernel_spmd

F32 = mybir.dt.float32
F16 = mybir.dt.float16

B, S, D, H, DH = 2, 2048, 1024, 16, 64
HPC = 4          # heads per core
NCORES = 8
KT = S // 128    # 16 key tiles of 128
QS = S // 512    # 4 query strips of 512
DKT = D // 128   # 8 contraction tiles for the projections


def _to_f16(x):
    return np.ascontiguousarray(x).astype(np.float16)


def _build(spec, uregw, debug=False):
    """spec[qs][ki] is ('s',) skip | ('f',) full | ('p', pref, regw, mid).

    pref = leading fully-masked query columns (sliced off entirely),
    regw = width of the mixed region right after pref (mask-multiplied),
    mid  = index into the unique mask tiles; uregw[mid] is that tile's width.
    """
    nc = bacc.Bacc()

    mw = sum(uregw) if uregw else 1
    # inputs are pre-tiled partition-major; names: x tokens 0-511 (kt 0-1 /
    # kt 2-7), x tokens 512-2047 (halves), wq/k chunks, wv, wo, mask regions
    xta = nc.dram_tensor("xta", [128, 2, 512], F16, kind="ExternalInput")
    xtb = nc.dram_tensor("xtb", [128, 3, 512], F16, kind="ExternalInput")
    xtc = nc.dram_tensor("xtc", [128, 3, 512], F16, kind="ExternalInput")
    xra = nc.dram_tensor("xra", [128, DKT, 768], F16, kind="ExternalInput")
    xrb = nc.dram_tensor("xrb", [128, DKT, 768], F16, kind="ExternalInput")
    wqa = nc.dram_tensor("wqa", [128, 2, 512], F16, kind="ExternalInput")
    wqb = nc.dram_tensor("wqb", [128, 3, 512], F16, kind="ExternalInput")
    wqc = nc.dram_tensor("wqc", [128, 3, 512], F16, kind="ExternalInput")
    wv = nc.dram_tensor("wv", [128, DKT, 256], F16, kind="ExternalInput")
    wo = nc.dram_tensor("wo", [128, 2, D], F16, kind="ExternalInput")
    maskp = nc.dram_tensor("maskp", [128, mw], F16, kind="ExternalInput")
    out = nc.dram_tensor("out", [128, 4 * QS, D], F16, kind="ExternalOutput")
    if debug:
        dbg = {
            "d_xt0": nc.dram_tensor("d_xt0", [128, DKT, 512], F16, kind="ExternalOutput"),
            "d_xtr": nc.dram_tensor("d_xtr", [128, DKT, S - 512], F16, kind="ExternalOutput"),
            "d_wqk": nc.dram_tensor("d_wqk", [128, DKT, 512], F16, kind="ExternalOutput"),
            "d_wv": nc.dram_tensor("d_wv", [128, DKT, 256], F16, kind="ExternalOutput"),
            "d_wo": nc.dram_tensor("d_wo", [128, 2, D], F16, kind="ExternalOutput"),
            "d_vext": nc.dram_tensor("d_vext", [128, KT * HPC * 65], F16, kind="ExternalOutput"),
            "d_ot0": nc.dram_tensor("d_ot0", [128, S], F16, kind="ExternalOutput"),
        }

    with tile.TileContext(nc) as tc, ExitStack() as top:
        persist = top.enter_context(tc.tile_pool(name="persist", bufs=1))

        # ---- persistent tiles ----
        # qk[ct]: transposed projections [proj-col, token]; ct 0-1 = q heads
        # (0,1),(2,3) scaled by 1/sqrt(dh) host-side; ct 2-3 = k heads.
        qk = [persist.tile([128, S], F16, name=f"qk{ct}", tag=f"qk{ct}") for ct in range(4)]
        # v_ext: one tile per key strip s; within it key-tile kt=s*4+j and
        # head h sit at column offset 260*j + 65*h, [128 tok, 64 dims + ones]
        # (split per strip so deps stay strip-local: dep tracking is
        # tile-granular, and a filler v-cast must not serialize against
        # attention PV reads of other strips)
        v_ext = [
            persist.tile([128, 4 * HPC * 65], F16, name=f"vx{s}", tag=f"vx{s}")
            for s in range(QS)
        ]
        # ot: per (tile t = head pair, strip): [128 head-dims, 512 tokens]
        ot = [
            [
                persist.tile([128, 512], F16, name=f"ot{t}_{s}", tag=f"ot{t}_{s}")
                for s in range(QS)
            ]
            for t in range(2)
        ]
        wo_t = persist.tile([128, 2, D], F16, tag="wo")
        mtiles = [
            persist.tile([128, uregw[m]], F16, name=f"mt{m}", tag=f"mt{m}")
            for m in range(len(uregw))
        ]
        osb = [
            persist.tile([128, 4, D], F16, name=f"osb{i}", tag=f"osb{i}")
            for i in range(2)
        ]

        # phase-1 input tiles
        xw = top.enter_context(tc.tile_pool(name="xw", bufs=1))
        xt0 = xw.tile([128, DKT, 512], F16, tag="xt0")
        xtr = xw.tile([128, DKT, S - 512], F16, tag="xtr")
        wqk_t = xw.tile([128, DKT, 512], F16, tag="wqk")
        wv_t = xw.tile([128, DKT, 256], F16, tag="wv")

        # ---- input DMAs: big descriptors, priority-ordered, 3 engines ----
        # ones columns of v_ext, generated on-device (a DMA here costs 8192
        # 2-byte descriptors that jam the queues for ~15us)
        for s in range(QS):
            nc.gpsimd.memset(
                v_ext[s][:].rearrange("p (g c) -> p g c", c=65)[:, :, 64:65], 1.0
            )
        # the 3MB x strips are dispatched from gpsimd AFTER the kt5-7 and wv
        # chunks: descriptors drain the queues FIFO, so anything dispatched
        # earlier than the weight chunks would delay the group-A matmuls
        nc.sync.dma_start(wqk_t[:, 0:2, :], wqa[:])
        nc.scalar.dma_start(xt0[:, 0:2, :], xta[:])
        nc.sync.dma_start(wqk_t[:, 2:5, :], wqb[:])
        nc.scalar.dma_start(xt0[:, 2:5, :], xtb[:])
        nc.sync.dma_start(wqk_t[:, 5:8, :], wqc[:])
        nc.scalar.dma_start(xt0[:, 5:8, :], xtc[:])
        nc.gpsimd.dma_start(wv_t[:], wv[:])
        nc.gpsimd.dma_start(xtr[:, :, 0:768], xra[:])
        nc.gpsimd.dma_start(xtr[:, :, 768:1536], xrb[:])
        nc.scalar.dma_start(wo_t[:], wo[:])
        moff = 0
        for m in range(len(uregw)):
            nc.sync.dma_start(mtiles[m][:], maskp[:, moff : moff + uregw[m]])
            moff += uregw[m]

        def xslice(kt, lo, hi):
            # columns [lo, hi) of the logical xT tile kt
            if hi <= 512:
                return xt0[:, kt, lo:hi]
            return xtr[:, kt, lo - 512 : hi - 512]

        # ---- phase 1 head: q/k for strip 0, kt-outer so matmuls start on
        # the first DMA chunk ----
        with ExitStack() as pha:
            psA = pha.enter_context(tc.tile_pool(name="psA", bufs=1, space="PSUM"))
            pa = [
                psA.tile([128, 512], F32, name=f"pa{ct}", tag=f"pa{ct}")
                for ct in range(4)
            ]
            for kt in range(DKT):
                for ct in range(4):
                    nc.tensor.matmul(
                        pa[ct][:],
                        wqk_t[:, kt, 128 * ct : 128 * ct + 128],
                        xt0[:, kt, :],
                        start=(kt == 0),
                        stop=(kt == DKT - 1),
                    )
            for ct in range(4):
                nc.vector.tensor_copy(qk[ct][:, 0:512], pa[ct][:])

        # ---- phase 2: attention, with the remaining projection work
        # (v tiles, q/k strips 1-3, per-strip output projections) emitted as
        # PE filler between attention tile-groups.  The exp() stream on the
        # scalar engine paces attention; the filler keeps the PE busy so the
        # two run concurrently instead of serializing. ----
        with ExitStack() as ph2:
            ptp = ph2.enter_context(tc.tile_pool(name="pt", bufs=4))
            nrm = ph2.enter_context(tc.tile_pool(name="nrm", bufs=3))
            ps_st = ph2.enter_context(
                tc.tile_pool(name="ps_st", bufs=2, space="PSUM")
            )
            ps_o = ph2.enter_context(tc.tile_pool(name="ps_o", bufs=2, space="PSUM"))
            fillp = ph2.enter_context(tc.tile_pool(name="fillp", bufs=2, space="PSUM"))

            def emit_v(st, eng=None):
                # v natural: psum[tok, head*64+d] = xT_tile.T @ wv_tile
                ps = fillp.tile([128, 512], F32, tag="fill")
                for kt in range(DKT):
                    nc.tensor.matmul(
                        ps[:, 0:256],
                        xslice(kt, 128 * st, 128 * st + 128),
                        wv_t[:, kt, :],
                        start=(kt == 0),
                        stop=(kt == DKT - 1),
                    )
                j = st % 4
                dst = v_ext[st // 4][:, 260 * j : 260 * j + 260].rearrange(
                    "p (h c) -> p h c", c=65
                )[:, :, 0:64]
                nc.vector.tensor_copy(
                    dst, ps[:, 0:256].rearrange("p (h c) -> p h c", c=64)
                )

            def emit_qk(ss, ct, eng=None):
                ps = fillp.tile([128, 512], F32, tag="fill")
                for kt in range(DKT):
                    nc.tensor.matmul(
                        ps[:],
                        wqk_t[:, kt, 128 * ct : 128 * ct + 128],
                        xtr[:, kt, 512 * ss - 512 : 512 * ss],
                        start=(kt == 0),
                        stop=(kt == DKT - 1),
                    )
                nc.vector.tensor_copy(qk[ct][:, 512 * ss : 512 * ss + 512], ps[:])

            def emit_oproj(qs, sti, oc, eng=None):
                ob = osb[qs % 2]
                pop = fillp.tile([128, 512], F32, tag="fill")
                for t in range(2):
                    nc.tensor.matmul(
                        pop[:],
                        ot[t][qs][:, 128 * sti : 128 * sti + 128],
                        wo_t[:, t, 512 * oc : 512 * oc + 512],
                        start=(t == 0),
                        stop=(t == 1),
                    )
                dst = ob[:, sti, 512 * oc : 512 * oc + 512]
                if eng == "scalar":
                    # tail-only: the exp stream is done there, so the scalar
                    # engine can halve the cast pacing of the output chain
                    nc.scalar.copy(dst, pop[:])
                else:
                    nc.vector.tensor_copy(dst, pop[:])
                if sti == 3 and oc == 1:
                    nc.sync.dma_start(out[:, 4 * qs : 4 * qs + 4, :], ob[:])

            def mk(f, *a):
                return lambda **kw: f(*a, **kw)

            # v for keys 0-511 must precede attention strip 0
            for st in range(4):
                emit_v(st)

            # filler due within attention strip qs (deps ready by then;
            # results needed only by strip qs+1)
            fills = {
                0: [mk(emit_qk, 1, ct) for ct in range(4)]
                + [mk(emit_v, st) for st in range(4, 8)],
                1: [mk(emit_qk, 2, ct) for ct in range(4)]
                + [mk(emit_v, st) for st in range(8, 12)]
                + [mk(emit_oproj, 0, sti, oc) for sti in range(4) for oc in range(2)],
                2: [mk(emit_qk, 3, ct) for ct in range(4)]
                + [mk(emit_v, st) for st in range(12, 16)]
                + [mk(emit_oproj, 1, sti, oc) for sti in range(4) for oc in range(2)],
                3: [mk(emit_oproj, 2, sti, oc) for sti in range(4) for oc in range(2)],
            }

            for qs in range(QS):
                partials = []   # (ki, pref, regw, mid)
                valids = []
                for ki in range(KT):
                    st = spec[qs][ki]
                    if st[0] == "f":
                        valids.append(ki)
                    elif st[0] == "p":
                        partials.append((ki, st[1], st[2], st[3]))
                # the PV accumulation must start with a full-width tile
                partials.sort(key=lambda t: t[1])
                assert (not partials) or partials[0][1] == 0, (
                    "need a full-width partial tile to open the accumulation"
                )
                # tile groups of <=2, partials first (j0 opens accumulation)
                groups = [
                    [(ki, pref, regw, mid) for ki, pref, regw, mid in partials[i : i + 2]]
                    for i in range(0, len(partials), 2)
                ]
                groups += [
                    [(ki, 0, 0, -1) for ki in valids[i : i + 2]]
                    for i in range(0, len(valids), 2)
                ]
                n_tiles = len(partials) + len(valids)
                fq = fills[qs]
                # pace filler across the strip's group slots, reserving a few
                # units to cover the end-of-strip normalize latency
                n_slots = HPC * (len(groups) + 1)
                reserve = 4 if qs == QS - 1 else 0
                stride = max(1, n_slots // max(len(fq) - reserve, 1))
                slot = 0

                for h in range(HPC):
                    qT = qk[h // 2][64 * (h % 2) : 64 * (h % 2) + 64, :]
                    kT = qk[2 + h // 2][64 * (h % 2) : 64 * (h % 2) + 64, :]
                    po = ps_o.tile([65, 512], F32, tag="po")

                    # scores/exp for group g run one step ahead of PV for
                    # group g-1 so the PE never sits waiting on exp
                    pending = None
                    npv = 0
                    for g in range(len(groups) + 1):
                        jobs = None
                        if g < len(groups):
                            grp = groups[g]
                            pst = ps_st.tile([128, 1024], F32, tag="pst")
                            pt = ptp.tile([128, 1024], F16, tag="pt")
                            jobs = []
                            for j, (ki, pref, regw, mid) in enumerate(grp):
                                w = 512 - pref
                                nc.tensor.matmul(
                                    pst[:, 512 * j : 512 * j + w],
                                    kT[:, 128 * ki : 128 * ki + 128],
                                    qT[:, 512 * qs + pref : 512 * qs + 512],
                                    start=True,
                                    stop=True,
                                )
                                jobs.append((pt, 512 * j, w, ki, pref))
                            w0 = 512 - grp[0][1]
                            if len(grp) == 1 or w0 == 512:
                                # contiguous region: single exp
                                wlast = 512 * (len(grp) - 1) + 512 - grp[-1][1]
                                nc.scalar.activation(
                                    pt[:, 0:wlast],
                                    pst[:, 0:wlast],
                                    mybir.ActivationFunctionType.Exp,
                                )
                            else:
                                for j, (ki, pref, regw, mid) in enumerate(grp):
                                    w = 512 - pref
                                    nc.scalar.activation(
                                        pt[:, 512 * j : 512 * j + w],
                                        pst[:, 512 * j : 512 * j + w],
                                        mybir.ActivationFunctionType.Exp,
                                    )
                            for j, (ki, pref, regw, mid) in enumerate(grp):
                                if mid >= 0:
                                    nc.vector.tensor_mul(
                                        pt[:, 512 * j : 512 * j + regw],
                                        pt[:, 512 * j : 512 * j + regw],
                                        mtiles[mid][:],
                                    )
                        if (
                            fq
                            and len(fq) > reserve
                            and slot % stride == stride - 1
                        ):
                            fq.pop(0)()
                        slot += 1
                        if pending is not None:
                            for pt_, off, w, ki, pref in pending:
                                vcol = 260 * (ki % 4) + 65 * h
                                nc.tensor.matmul(
                                    po[:, pref : pref + w],
                                    v_ext[ki // 4][:, vcol : vcol + 65],
                                    pt_[:, off : off + w],
                                    start=(npv == 0),
                                    stop=(npv == n_tiles - 1),
                                )
                                npv += 1
                        pending = jobs

                    if h == HPC - 1:
                        # flush leftover filler BEFORE the normalize chain:
                        # anything emitted after it inherits a wait on the
                        # chain's final vector op and sits out ~3us
                        nflush = 0
                        while fq:
                            fq.pop(0)(eng="scalar" if nflush % 2 else None)
                            nflush += 1
                    # normalize: row 64 of po is the softmax denominator
                    # (copy to SBUF first: the custom-DVE reciprocal misreads
                    # PSUM operands on hardware)
                    rden = nrm.tile([1, 512], F32, tag="rden")
                    nc.vector.tensor_copy(rden[:], po[64:65, :])
                    rrec = nrm.tile([1, 512], F32, tag="rrec")
                    nc.vector.reciprocal_approx_fast(rrec[:], rden[:])
                    rb = nrm.tile([64, 512], F32, tag="rb")
                    nc.gpsimd.partition_broadcast(rb[:], rrec[:])
                    nc.vector.tensor_mul(
                        ot[h // 2][qs][64 * (h % 2) : 64 * (h % 2) + 64, :],
                        po[0:64, :],
                        rb[:],
                    )

                # flush filler still due before the next strip starts
                while fq:
                    fq.pop(0)()

            # strip 3's output projection runs after its last head
            for sti in range(4):
                for oc in range(2):
                    emit_oproj(3, sti, oc, eng="scalar" if oc else None)

        if debug:
            nc.sync.dma_start(dbg["d_xt0"][:], xt0[:])
            nc.sync.dma_start(dbg["d_xtr"][:], xtr[:])
            nc.sync.dma_start(dbg["d_wqk"][:], wqk_t[:])
            nc.sync.dma_start(dbg["d_wv"][:], wv_t[:])
            nc.sync.dma_start(dbg["d_wo"][:], wo_t[:])
            for s in range(QS):
                nc.sync.dma_start(
                    dbg["d_vext"][:, 1040 * s : 1040 * s + 1040], v_ext[s][:]
                )
                nc.sync.dma_start(
                    dbg["d_ot0"][:, 512 * s : 512 * s + 512], ot[0][s][:]
                )

    nc.finalize()
    return nc


_cache = {}


def _get_nc(key):
    if key not in _cache:
        spec, uregw = key
        _cache[key] = _build([list(r) for r in spec], list(uregw))
    return _cache[key]


def _tile_km(a):
    """[K*128, w] -> [128, K, w] partition-major contiguous."""
    k1, w = a.shape
    return np.ascontiguousarray(
        a.reshape(k1 // 128, 128, w).transpose(1, 0, 2)
    )


def _prepare(x, mask, w_qkv, w_out):
    """Host-side sharding. Returns (cache_key, in_maps)."""
    scale = 1.0 / np.sqrt(DH)

    # classify score tiles from the actual mask, merged across batches so one
    # SPMD program works for all cores.  keep[k, q] = 1 iff key k visible to
    # query q.  A tile is skip if fully masked in every batch, full if fully
    # valid in every batch, else partial with a sliced prefix + mixed region.
    keeps = [(mask[b] != 0).T.astype(np.float32) for b in range(B)]  # [k, q]
    keep_any = np.maximum.reduce(keeps)   # visible in some batch
    keep_all = np.minimum.reduce(keeps)   # visible in every batch

    uniq = {}
    uregw = []
    umask = []
    spec = []
    for qs in range(QS):
        row = []
        for ki in range(KT):
            blk_any = keep_any[128 * ki : 128 * ki + 128, 512 * qs : 512 * qs + 512]
            blk_all = keep_all[128 * ki : 128 * ki + 128, 512 * qs : 512 * qs + 512]
            if blk_any.max() == 0.0:
                row.append(("s",))
                continue
            if blk_all.min() == 1.0:
                row.append(("f",))
                continue
            colm = blk_any.max(axis=0)   # col has any visible key
            colv = blk_all.min(axis=0)   # col fully valid
            nz = np.nonzero(colm)[0]
            pref = int(nz[0]) if len(nz) else 512
            mixed = np.nonzero(colv == 0)[0]
            end = int(mixed[-1]) + 1 if len(mixed) else pref
            regw = max(end - pref, 1)
            regs = tuple(
                k[128 * ki : 128 * ki + 128, 512 * qs + pref : 512 * qs + pref + regw]
                .astype(np.float16)
                .tobytes()
                for k in keeps
            )
            if regs not in uniq:
                uniq[regs] = len(uregw)
                uregw.append(regw)
                umask.append(
                    [
                        np.frombuffer(r, np.float16).reshape(128, regw)
                        for r in regs
                    ]
                )
            row.append(("p", pref, regw, uniq[regs]))
        spec.append(tuple(row))
    key = (tuple(spec), tuple(uregw))

    in_maps = []
    for c in range(NCORES):
        b, g = c // 4, c % 4
        heads = range(4 * g, 4 * g + 4)
        xT = _tile_km(_to_f16(x[b].T))            # [128, 8, 2048]
        wq = np.concatenate(
            [w_qkv[:, 64 * h : 64 * h + 64] for h in heads], axis=1
        ) * scale
        wk = np.concatenate(
            [w_qkv[:, D + 64 * h : D + 64 * h + 64] for h in heads], axis=1
        )
        wvv = np.concatenate(
            [w_qkv[:, 2 * D + 64 * h : 2 * D + 64 * h + 64] for h in heads], axis=1
        )
        woo = np.concatenate(
            [w_out[64 * h : 64 * h + 64, :] for h in heads], axis=0
        )
        wqk = _tile_km(_to_f16(np.concatenate([wq, wk], axis=1)))  # [128, 8, 512]
        if umask:
            mk = np.concatenate([r[b] for r in umask], axis=1).astype(np.float16)
        else:
            mk = np.zeros((128, 1), np.float16)
        in_maps.append(
            {
                "xta": np.ascontiguousarray(xT[:, 0:2, 0:512]),
                "xtb": np.ascontiguousarray(xT[:, 2:5, 0:512]),
                "xtc": np.ascontiguousarray(xT[:, 5:8, 0:512]),
                "xra": np.ascontiguousarray(xT[:, :, 512:1280]),
                "xrb": np.ascontiguousarray(xT[:, :, 1280:2048]),
                "wqa": np.ascontiguousarray(wqk[:, 0:2, :]),
                "wqb": np.ascontiguousarray(wqk[:, 2:5, :]),
                "wqc": np.ascontiguousarray(wqk[:, 5:8, :]),
                "wv": _tile_km(_to_f16(wvv)),
                "wo": _tile_km(_to_f16(np.ascontiguousarray(woo))),
                "maskp": np.ascontiguousarray(mk),
            }
        )
    return key, in_maps


def _unshuffle_out(o):
    """[128, 16, D] tile-major kernel output -> [S, D]."""
    return np.ascontiguousarray(o.transpose(1, 0, 2)).reshape(S, D)


def _run(x, mask, w_qkv, w_out, trace=False, trace_cores=None):
    key, in_maps = _prepare(x, mask, w_qkv, w_out)
    nc = _get_nc(key)
    res = run_bass_kernel_spmd(
        nc,
        in_maps,
        core_ids=list(range(NCORES)),
        trace=trace,
        trace_cores=trace_cores,
    )
    outs = np.stack(
        [
            sum(
                _unshuffle_out(res.results[4 * b + g]["out"].astype(np.float32))
                for g in range(4)
            )
            for b in range(B)
        ]
    )
    return outs.astype(np.float32), res


def kernel(x, mask, w_qkv, w_out):
    x = np.asarray(x, np.float32)
    mask = np.asarray(mask)
    w_qkv = np.asarray(w_qkv, np.float32)
    w_out = np.asarray(w_out, np.float32)
    out, _ = _run(x, mask, w_qkv, w_out)
    return out
